# revision 26
# baseline (speedup 1.0000x reference)
"""CircleLoss (N=8192, D=128, C=512, m=0.25, gamma=64) on 8 Trainium2 cores.

Math (forward, stop_gradient is identity):
  x = L2-normalize rows;  s_ij = x_i . x_j;  mask = same-class (incl diag)
  -logit_p = (1.25 - s)(s - 0.75)*64 = 4 - 64 (s-1)^2        (ap>0 always since s<=1)
  logit_n  = relu(s - 0.25) * (s - 0.25) * 64 = 64 relu(s-0.25)^2
  lp = logsumexp_pos(-logit_p); ln = logsumexp_neg(logit_n)
  loss = mean softplus(ln + lp) = mean log(1 + S_n * S_p)
where S_p = sum_pos exp(4 - 64 (s-1)^2),  S_n = sum_neg exp(64 relu(s-0.25)^2).
No max-subtraction needed: terms bounded by e^4 (pos) / e^36 (neg), sums fit fp32.

Strategy: host sorts rows by class (mean over rows is permutation invariant), so
the positive set of every row is a contiguous window of columns lying within
+-64 of the row's own block.  Each of the 8 cores handles 1024 rows:
  - normalize all 8192 embeddings on device (redundantly per core)
  - sim block [1024, 8192] via PE (bf16), tile [128, 512]
  - S_n:  full row-sum of w = exp(64 relu(s-.25)^2) via ACT-exp accumulate,
          minus the in-window w-sum computed on a small [128, 256] band
  - S_p:  masked band sum of p = exp(4 - 64 (s-1)^2)
  - loss rows = ln(1 + S_p * S_n) -> host mean.
"""

import functools

import numpy as np
import ml_dtypes

import concourse.bass as bass
import concourse.tile as tile
from concourse import mybir
from concourse.tile import ScopedClock
from concourse.bass_utils import run_bass_kernel_spmd

F32 = mybir.dt.float32
BF16 = mybir.dt.bfloat16
ALU = mybir.AluOpType
AF = mybir.ActivationFunctionType


def _register_relu2_cap_op():
    """Register a custom DVE op: out = relu(min(in0 + c0, c1))^2.

    Fuses the whole capped-relu-square pass (PSUM sim tile -> bf16 q tile)
    into one DVE instruction.  Appended to concourse.dve_ops.OPS at import
    so compile-side table gen and CoreSim both see it."""
    import concourse.dve_ops as dve_ops
    from concourse.dve_spec import Spec, Src0, C0, C1, relu, minn, sq, lower
    from concourse.dve_uop import DveOpSpec

    name = "RELU2_MINCAP_ANT"
    if name in dve_ops._SUB_OPCODE_FOR_NAME:
        return next(op for op in dve_ops.OPS if op.name == name)

    def _ref(in0, in1, c0, c1, c2):
        v = np.minimum(in0.astype(np.float32) + c0, c1)
        return np.maximum(v, 0) ** 2

    spec = Spec(body=sq(relu(minn(Src0 + C0, C1))), reference=_ref)
    row = dve_ops._CUSTOM_DVE_ROW_BASE + len(dve_ops.OPS)
    shas = {}
    for ver in ("v3", "v4"):
        so = DveOpSpec(name=name, opcode=row, uops=lower(spec, ver=ver), rd1_en=False)
        shas[ver] = so.sha(ver)
    op = dve_ops.DveOp(name, spec, subdim=False, uops_sha=shas)
    dve_ops.OPS.append(op)
    dve_ops.CUSTOM_DVE_SPECS[name] = spec
    dve_ops._SUB_OPCODE_FOR_NAME[name] = row
    return op


RELU2_MINCAP = _register_relu2_cap_op()

N, D, C = 8192, 128, 512
NCORES = 8
ROWS = N // NCORES            # 1024 rows per core
ICH = ROWS // 128             # 8 i-chunks of 128 rows
JT = N // 512                 # 16 j-tiles of 512 cols
CAP = 0.4                     # cap on (s - 0.25); see pass-1 comment
ACT_TILES = ()                # per-chunk j-tiles whose pass-1 runs on ScalarE
BPAD = 64                     # band padding (max class size asserted <= 64)
BW = 128 + 2 * BPAD           # 256-wide positive window per i-chunk
DCOLS = ROWS + 2 * BPAD       # 1152 band columns per core


class SplitWaitTC(tile.TileContext):
    """TileContext whose final drain splits sem-waits one-per-instruction.

    This walrus build rejects instructions carrying more than ~2 sync wait
    commands ("Too many sync wait commands"); the stock kernel-tail drain
    carries one wait per live proc.
    """

    MAX_WAITS = 1

    def _drain_and_barrier(self, tick_clock, wait_clock):
        drain_inst = self.nc.sync.drain()
        wait_clock.add_sem_waits(
            drain_inst.ins, ScopedClock({None: tick_clock.global_clock})
        )
        si = drain_inst.ins.sync_info
        waits = list(si.on_wait) if si and si.on_wait else []
        if len(waits) > self.MAX_WAITS:
            si.on_wait = waits[: self.MAX_WAITS]
            rest = waits[self.MAX_WAITS :]
            while rest:
                extra = self.nc.sync.drain()
                chunk, rest = rest[: self.MAX_WAITS], rest[self.MAX_WAITS :]
                extra.ins.sync_info = mybir.SyncInfo(on_wait=chunk, on_update=[])
            # (tail stays drains: they must actually drain the queues)
        self.nc.all_engine_barrier()
        popped = self.nc._tile_sem_poison_stack.pop()
        assert popped is self._sem_poison
        # clear_and_free_semaphores emits EVENT_SEMAPHORE_RANGE_CLEAR, which
        # this walrus build rejects ("ISA wrong length").  Skip the runtime
        # sem reset: each PJRT executable instantiation reloads the NEFF,
        # which re-initializes semaphore state, and this kernel is executed
        # once per load.  Keep the compile-time bookkeeping only.
        sems = list(self.sems.allocated().values())
        if sems:
            sem_nums = [s.num for s in sems]
            self.nc._state.prepend_free_semaphores(sem_nums)
            for poison_set in self.nc._tile_sem_poison_stack:
                poison_set.update(sem_nums)
        self.nc.all_engine_barrier()


def _split_excess_waits(nc, max_waits=1):
    """Walrus rejects >~2 sync waits on one instruction; move excess waits
    onto NoOp instructions inserted just before the offender (same engine,
    same basic block => same per-engine program order)."""
    nop_id = [0]
    for fn in nc.m.functions:
        for blk in fn.blocks:
            insts = blk.instructions
            out = []
            changed = False
            for inst in insts:
                si = inst.sync_info
                waits = list(si.on_wait) if si and si.on_wait else []
                if len(waits) > max_waits:
                    rest = waits[:-max_waits]
                    si.on_wait = waits[-max_waits:]
                    while rest:
                        chunk, rest = rest[:max_waits], rest[max_waits:]
                        nop = mybir.InstEventSemaphore(
                            name=f"I-waitsplit-{nop_id[0]}", ins=[], outs=[]
                        )
                        nop_id[0] += 1
                        nop.engine = inst.engine
                        nop.sync_info = mybir.SyncInfo(on_wait=chunk, on_update=[])
                        nc.register_instruction(nop, overwrite=True)
                        out.append(nop)
                    changed = True
                out.append(inst)
            if changed:
                blk.instructions = out


def _chunks_of(ncols, width=512):
    out = []
    off = 0
    while off < ncols:
        w = min(width, ncols - off)
        out.append((off, w))
        off += w
    return out


def _emit_normalize(nc, pA, psA, ones128, ones1, dram_name, ncols, dst):
    """dst[:, :ncols] (bf16) = column-L2-normalized copy of DRAM [128, ncols].

    Columns are embeddings (x.T layout).  norm^2 via ACT-square + PE ones-
    matmul (column sums broadcast over partitions), rsqrt via ACT-sqrt + DVE
    reciprocal on a compact [128, ncols/128] layout, then one PE broadcast
    matmul + DVE multiply per 512-chunk.
    """
    src = nc.dram_tensor(dram_name, [128, ncols], F32, kind="ExternalInput")
    cp = ncols // 128  # compact cols; value j lives at [j % 128, j // 128]

    xt = pA.tile([128, ncols], F32, tag=f"xt_{dram_name}")
    for off, w in _chunks_of(ncols, 2048):
        nc.sync.dma_start(out=xt[:, off : off + w], in_=src[:, off : off + w])

    x2 = pA.tile([128, ncols], BF16, tag=f"x2_{dram_name}")
    for off, w in _chunks_of(ncols, 2048):
        nc.scalar.activation(x2[:, off : off + w], xt[:, off : off + w], AF.Square)

    n2row = pA.tile([1, ncols], F32, tag=f"n2row_{dram_name}")
    for off, w in _chunks_of(ncols, 1024):
        n2ps = psA.tile([128, 1024], F32, tag="n2ps")
        for o2 in range(0, w, 512):
            w2 = min(512, w - o2)
            nc.tensor.matmul(
                n2ps[:, o2 : o2 + w2],
                ones128,
                x2[:, off + o2 : off + o2 + w2],
                start=True,
                stop=True,
            )
        # all psum partitions hold the same column sums; evacuate row 0
        nc.scalar.copy(n2row[:, off : off + w], n2ps[0:1, :w])

    # reshape row into p-major compact tile: n2pc[p, c] = n2(p*cp + c)
    n2pc = pA.tile([128, cp], F32, tag=f"n2pc_{dram_name}")
    src_ap = bass.AP(
        tensor=n2row.tensor,
        offset=n2row.offset,
        ap=[[n2row.ap[0][0], 1], [1, ncols]],
    )
    nc.sync.dma_start(out=n2pc[:, :], in_=src_ap)

    nrm = pA.tile([128, cp], F32, tag=f"nrm_{dram_name}")
    nc.scalar.activation(nrm, n2pc, AF.Sqrt)
    rn = pA.tile([128, cp], F32, tag=f"rn_{dram_name}")
    nc.vector.reciprocal(rn, nrm)
    rn_bf = pA.tile([128, cp], BF16, tag=f"rnbf_{dram_name}")
    nc.vector.tensor_copy(rn_bf, rn)

    # compact -> row [1, ncols]: row[p*cp + c] = rn_bf[p, c]
    rnrow = pA.tile([1, ncols], BF16, tag=f"rnrow_{dram_name}")
    dst_ap = bass.AP(
        tensor=rnrow.tensor,
        offset=rnrow.offset,
        ap=[[rnrow.ap[0][0], 1], [1, ncols]],
    )
    nc.sync.dma_start(out=dst_ap, in_=rn_bf[:, :])

    for off, w in _chunks_of(ncols, 1024):
        rnb = psA.tile([128, 1024], F32, tag="rnb")
        for o2 in range(0, w, 512):
            w2 = min(512, w - o2)
            nc.tensor.matmul(
                rnb[:, o2 : o2 + w2],
                ones1,
                rnrow[:, off + o2 : off + o2 + w2],
                start=True,
                stop=True,
            )
        nc.vector.tensor_tensor(
            out=dst[:, off : off + w],
            in0=xt[:, off : off + w],
            in1=rnb[:, :w],
            op=ALU.mult,
        )


@functools.lru_cache(maxsize=1)
def _build_program():
    nc = bass.Bass()

    mask_dram = nc.dram_tensor("mask", [128, ICH * BW], BF16, kind="ExternalInput")
    loss_dram = nc.dram_tensor("loss", [128, ICH], F32, kind="ExternalOutput")
    sn_dram = nc.dram_tensor("dbg_sn", [128, ICH], F32, kind="ExternalOutput")
    sp_dram = nc.dram_tensor("dbg_sp", [128, ICH], F32, kind="ExternalOutput")
    rsum_dram = nc.dram_tensor("dbg_rsum", [128, ICH], F32, kind="ExternalOutput")
    ww_dram = nc.dram_tensor("dbg_ww", [128, ICH], F32, kind="ExternalOutput")

    with SplitWaitTC(nc) as tc:
        persist = tc.tile_pool(name="persist", bufs=1)
        with persist as pp:
            xhatT = pp.tile([128, N], BF16)
            xhatD = pp.tile([128, DCOLS], BF16)
            maskT = pp.tile([128, ICH * BW], BF16)
            nc.sync.dma_start(out=maskT, in_=mask_dram[:, :])
            rsum = pp.tile([128, ICH], F32)
            Ww = pp.tile([128, ICH], F32)
            Sp = pp.tile([128, ICH], F32)
            ones128 = pp.tile([128, 128], BF16)
            nc.vector.memset(ones128, 1.0)
            ones1 = pp.tile([1, 128], BF16)
            nc.vector.memset(ones1, 1.0)
            bias_m1 = pp.tile([128, 1], F32)
            nc.vector.memset(bias_m1, -1.0)
            bias_p4 = pp.tile([128, 1], F32)
            nc.vector.memset(bias_p4, 4.0)
            bias_p1 = pp.tile([128, 1], F32)
            nc.vector.memset(bias_p1, 1.0)
            bias_0 = pp.tile([128, 1], F32)
            nc.vector.memset(bias_0, 0.0)
            bias_mq = pp.tile([128, 1], F32)
            nc.vector.memset(bias_mq, -0.25)
            bias_cap = pp.tile([128, 1], F32)
            nc.vector.memset(bias_cap, CAP)

            with (
                tc.tile_pool(name="phaseA", bufs=1) as pA,
                tc.tile_pool(name="psA", bufs=2, space="PSUM") as psA,
            ):
                _emit_normalize(nc, pA, psA, ones128, ones1, "xdT", DCOLS, xhatD)
                _emit_normalize(nc, pA, psA, ones128, ones1, "xT", N, xhatT)

            with (
                tc.tile_pool(name="qw", bufs=3) as qw,
                tc.tile_pool(name="rp", bufs=3) as rp,
                tc.tile_pool(name="bp", bufs=2) as bp,
                tc.tile_pool(name="psB", bufs=3, space="PSUM") as psB,
                tc.tile_pool(name="psC", bufs=2, space="PSUM") as psC,
            ):
                for k in range(ICH):
                    wts = xhatD[:, BPAD + 128 * k : BPAD + 128 * (k + 1)]
                    q = qw.tile([128, N], BF16, tag="q")
                    for t2 in range(JT // 2):
                        s_ps = psB.tile([128, 1024], F32, tag="s")
                        for h in range(2):
                            nc.tensor.matmul(
                                s_ps[:, 512 * h : 512 * (h + 1)],
                                wts,
                                xhatT[:, 1024 * t2 + 512 * h : 1024 * t2 + 512 * (h + 1)],
                                start=True,
                                stop=True,
                            )
                        # q = relu(min(s - 0.25, CAP))^2.
                        # CAP renders the in-window diagonal (s=1) as e^(64*CAP^2)
                        # instead of e^36 so S_n = rowsum - windowsum does not
                        # catastrophically cancel in fp32.  True negatives have
                        # s - 0.25 << CAP, so they are unaffected.
                        qslice = q[:, 1024 * t2 : 1024 * (t2 + 1)]
                        nc.vector._custom_dve(
                            RELU2_MINCAP, out=qslice, in0=s_ps, s0=-0.25, s1=CAP
                        )

                    # band: positives of rows [128k, 128k+128) live in
                    # band cols [128k, 128k + BW) of xhatD
                    sb = psC.tile([128, BW], F32, tag="sb")
                    nc.tensor.matmul(
                        sb, wts, xhatD[:, 128 * k : 128 * k + BW], start=True, stop=True
                    )
                    mk = maskT[:, BW * k : BW * (k + 1)]
                    v = bp.tile([128, BW], BF16, tag="v")
                    nc.scalar.activation(v, sb, AF.Square, bias=bias_m1, scale=1.0)
                    pb = bp.tile([128, BW], F32, tag="pb")
                    nc.scalar.activation(pb, v, AF.Exp, bias=bias_p4, scale=-64.0)
                    junk1 = bp.tile([128, BW], F32, tag="junk1")
                    nc.vector.scalar_tensor_tensor(
                        out=junk1,
                        in0=pb,
                        scalar=1.0,
                        in1=mk,
                        op0=ALU.mult,
                        op1=ALU.mult,
                        accum_out=Sp[:, k : k + 1],
                    )
                    qb = bp.tile([128, BW], BF16, tag="qb")
                    nc.vector._custom_dve(
                        RELU2_MINCAP, out=qb, in0=sb, s0=-0.25, s1=CAP
                    )
                    wb = bp.tile([128, BW], F32, tag="wb")
                    nc.scalar.activation(wb, qb, AF.Exp, scale=64.0)
                    junk2 = bp.tile([128, BW], F32, tag="junk2")
                    nc.vector.scalar_tensor_tensor(
                        out=junk2,
                        in0=wb,
                        scalar=1.0,
                        in1=mk,
                        op0=ALU.mult,
                        op1=ALU.mult,
                        accum_out=Ww[:, k : k + 1],
                    )

                    # full-row sum of w = exp(64 q) via ACT accumulate
                    W = qw.tile([128, N], BF16, tag="W")
                    nc.scalar.activation(
                        W, q, AF.Exp, scale=64.0, accum_out=rsum[:, k : k + 1]
                    )

                sn = pp.tile([128, ICH], F32)
                nc.vector.tensor_tensor(out=sn, in0=rsum, in1=Ww, op=ALU.subtract)
                z = pp.tile([128, ICH], F32)
                nc.vector.tensor_tensor(out=z, in0=sn, in1=Sp, op=ALU.mult)
                lossT = pp.tile([128, ICH], F32)
                nc.scalar.activation(lossT, z, AF.Ln, bias=bias_p1, scale=1.0)
                nc.sync.dma_start(out=loss_dram[:, :], in_=lossT)
                nc.sync.dma_start(out=sn_dram[:, :], in_=sn)
                nc.sync.dma_start(out=sp_dram[:, :], in_=Sp)
                nc.sync.dma_start(out=rsum_dram[:, :], in_=rsum)
                nc.sync.dma_start(out=ww_dram[:, :], in_=Ww)

    # fill instr bytes for InstCustomDveAnt (Bacc.compile does this; the
    # plain-Bass bass2jax path does not)
    mybir.codegen_inst_isa_subclasses(nc)
    _split_excess_waits(nc, max_waits=1)
    return nc


def _prepare_inputs(inputs, targets):
    x = np.asarray(inputs, dtype=np.float32)
    t = np.asarray(targets)
    perm = np.argsort(t, kind="stable")
    xs = x[perm]
    ts = t[perm]

    counts = np.bincount(ts.astype(np.int64), minlength=C)
    maxc = int(counts.max())
    assert maxc <= BPAD, f"class size {maxc} exceeds band padding {BPAD}"
    cstart = np.concatenate([[0], np.cumsum(counts)[:-1]])
    a = cstart[ts]            # window start per sorted row
    b = a + counts[ts]        # window end per sorted row

    xT = np.ascontiguousarray(xs.T)  # [128, N]

    in_maps = []
    for m in range(NCORES):
        base = ROWS * m
        idx = (base - BPAD + np.arange(DCOLS)) % N
        xdT = np.ascontiguousarray(xT[:, idx])

        # mask[p, k*BW + u] = 1 iff col (base + 128k - BPAD + u) in window of
        # row (base + 128k + p)
        kk = np.arange(ICH)[:, None, None]
        ppp = np.arange(128)[None, :, None]
        uu = np.arange(BW)[None, None, :]
        i_glob = base + 128 * kk + ppp
        j_unw = base + 128 * kk - BPAD + uu
        msk = (j_unw >= a[i_glob]) & (j_unw < b[i_glob])
        # windows must fit the band
        lo = a[base : base + ROWS] - base
        hi = b[base : base + ROWS] - base
        kloc = np.arange(ROWS) // 128
        assert (lo >= 128 * kloc - BPAD).all() and (hi <= 128 * kloc - BPAD + BW).all()
        mask = (
            msk.transpose(1, 0, 2).reshape(128, ICH * BW).astype(ml_dtypes.bfloat16)
        )
        in_maps.append(
            {
                "xT": xT.copy(),
                "xdT": xdT,
                "mask": mask,
            }
        )
    return in_maps


def run(inputs, targets, trace=False, tmpdir=None):
    nc = _build_program()
    in_maps = _prepare_inputs(inputs, targets)
    res = run_bass_kernel_spmd(
        nc, in_maps, core_ids=list(range(NCORES)), trace=trace, tmpdir=tmpdir
    )
    rows = []
    for r in res.results:
        lt = np.asarray(r["loss"])  # [128, ICH]; row i_loc = 128k + p at [p, k]
        rows.append(lt.T.reshape(-1))
    loss_rows = np.concatenate(rows)  # sorted order; mean is perm-invariant
    loss = np.float64(loss_rows.mean())
    return np.array(loss, dtype=np.float32), res


def kernel(inputs, targets):
    out, _ = run(inputs, targets)
    return out
